# revision 4
# baseline (speedup 1.0000x reference)
"""LSNN cell (LIF + adaptive-LIF) Trainium2 Bass kernel, 8-core data-parallel.

Math (per reference):
    new_b = db*b + z
    thr   = 0.4 + new_b*beta      (beta = 0 for first 1024 units, 1.6 for rest)
    i_t   = x@w_in + z@w_rec
    new_v = d*v + i_t - 0.4*z
    new_z = (r <= 0) * (new_v > thr)
    new_r = clip(r + 5*new_z - 1, 0, 5)   == max(r-1,0) + 4*new_z on the input domain
    new_out = k*out + new_z@w_out

Sharding: batch 4096 -> 8 cores x 512. Weights replicated.

Matmul precision: fp32 operands are split on the host into bf16 hi/lo pairs;
x@w_in uses 3 bf16 passes (hh + lh + hl), z@w_rec and new_z@w_out use 2
passes each (z and new_z are exactly representable in bf16). The -0.4*z term
is folded into w_rec's diagonal on the host. All accumulation is fp32 in PSUM.
"""

import numpy as np
import ml_dtypes

import concourse.bacc as bacc
import concourse.tile as tile
import concourse.mybir as mybir
from concourse import bass_utils
from concourse.alu_op_type import AluOpType as alu

BF16 = ml_dtypes.bfloat16

B, N_IN, UNITS, N_OUT = 4096, 1024, 2048, 1024
NCORES = 8
BS = B // NCORES            # 512 batch rows per core
P = 128                     # partition tile
BT = BS // P                # 4 batch tiles per core
NB = UNITS // 512           # 4 unit blocks of 512
KX = N_IN // P              # 8 k-tiles for x
KU = UNITS // P             # 16 k-tiles for units
OB = N_OUT // 512           # 2 output blocks

THR = 0.4
N_REF = 5.0
DECAY = float(np.exp(-1.0 / 20.0))
KAPPA = float(np.exp(-1.0 / 20.0))
DECAY_B = float(np.exp(-1.0 / 600.0))
BETA = 1.6

F32 = mybir.dt.float32
BF = mybir.dt.bfloat16

_cached_nc = None


def build():
    global _cached_nc
    if _cached_nc is not None:
        return _cached_nc
    nc = bacc.Bacc("TRN2", target_bir_lowering=False, debug=False,
                   num_devices=NCORES)

    # ---- per-core DRAM I/O ----
    xhT = nc.dram_tensor("xhT", [N_IN, BS], BF, kind="ExternalInput").ap()
    xlT = nc.dram_tensor("xlT", [N_IN, BS], BF, kind="ExternalInput").ap()
    zT = nc.dram_tensor("zT", [UNITS, BS], BF, kind="ExternalInput").ap()
    z_in = nc.dram_tensor("z", [BS, UNITS], F32, kind="ExternalInput").ap()
    v_in = nc.dram_tensor("v", [BS, UNITS], F32, kind="ExternalInput").ap()
    r_in = nc.dram_tensor("r", [BS, UNITS], F32, kind="ExternalInput").ap()
    b_in = nc.dram_tensor("b", [BS, UNITS], F32, kind="ExternalInput").ap()
    o_in = nc.dram_tensor("o", [BS, N_OUT], F32, kind="ExternalInput").ap()
    wih = nc.dram_tensor("wih", [N_IN, UNITS], BF, kind="ExternalInput").ap()
    wil = nc.dram_tensor("wil", [N_IN, UNITS], BF, kind="ExternalInput").ap()
    wrh = nc.dram_tensor("wrh", [UNITS, UNITS], BF, kind="ExternalInput").ap()
    wrl = nc.dram_tensor("wrl", [UNITS, UNITS], BF, kind="ExternalInput").ap()
    woh = nc.dram_tensor("woh", [UNITS, N_OUT], BF, kind="ExternalInput").ap()
    wol = nc.dram_tensor("wol", [UNITS, N_OUT], BF, kind="ExternalInput").ap()

    new_out = nc.dram_tensor("new_out", [BS, N_OUT], F32, kind="ExternalOutput").ap()
    new_z = nc.dram_tensor("new_z", [BS, UNITS], F32, kind="ExternalOutput").ap()
    new_v = nc.dram_tensor("new_v", [BS, UNITS], F32, kind="ExternalOutput").ap()
    new_r = nc.dram_tensor("new_r", [BS, UNITS], F32, kind="ExternalOutput").ap()
    new_b = nc.dram_tensor("new_b", [BS, UNITS], F32, kind="ExternalOutput").ap()

    xhT_r = xhT.rearrange("(k p) n -> k p n", p=P)
    xlT_r = xlT.rearrange("(k p) n -> k p n", p=P)
    zT_r = zT.rearrange("(k p) n -> k p n", p=P)
    wih_r = wih.rearrange("(k p) n -> k p n", p=P)
    wil_r = wil.rearrange("(k p) n -> k p n", p=P)
    wrh_r = wrh.rearrange("(k p) n -> k p n", p=P)
    wrl_r = wrl.rearrange("(k p) n -> k p n", p=P)
    woh_r = woh.rearrange("(k p) n -> k p n", p=P)
    wol_r = wol.rearrange("(k p) n -> k p n", p=P)

    with tile.TileContext(nc) as tc:
        with tc.tile_pool(name="const", bufs=1) as pc, \
             tc.tile_pool(name="w", bufs=10) as pw, \
             tc.tile_pool(name="wo", bufs=4) as pwo, \
             tc.tile_pool(name="act", bufs=4) as pa, \
             tc.tile_pool(name="act2", bufs=2) as pa2, \
             tc.tile_pool(name="tmp", bufs=3) as pt, \
             tc.tile_pool(name="outs", bufs=4) as po, \
             tc.tile_pool(name="nzp", bufs=4) as pnz, \
             tc.tile_pool(name="psA", bufs=4, space="PSUM") as ppsA, \
             tc.tile_pool(name="psB", bufs=4, space="PSUM") as ppsB, \
             tc.tile_pool(name="scr", bufs=1, space="DRAM") as pdram:

            # resident transposed activations
            XH = pc.tile([P, KX, BS], BF, name="XH")
            XL = pc.tile([P, KX, BS], BF, name="XL")
            ZT = pc.tile([P, KU, BS], BF, name="ZTt")
            for k in range(KX):
                nc.sync.dma_start(out=XH[:, k, :], in_=xhT_r[k])
                nc.sync.dma_start(out=XL[:, k, :], in_=xlT_r[k])
            for k in range(KU):
                nc.sync.dma_start(out=ZT[:, k, :], in_=zT_r[k])

            nzb = pdram.tile([BS, UNITS], BF, name="nzb")

            # ---------- phase A: i_t, state update, spikes ----------
            for nb in range(NB):
                ns = slice(nb * 512, (nb + 1) * 512)

                # per-(nb,bt) psum accumulators, filled k-outer/bt-inner so
                # each streamed weight tile dies after 4 bt uses
                ps = []
                for bt in range(BT):
                    p_ = ppsA.tile([P, 512], F32, name="p_", tag="ps")
                    ps.append(p_)

                for k in range(KX):
                    WIH = pw.tile([P, 512], BF, name="WIH", tag="wih")
                    WIL = pw.tile([P, 512], BF, name="WIL", tag="wil")
                    nc.sync.dma_start(out=WIH[:, :], in_=wih_r[k, :, ns])
                    nc.sync.dma_start(out=WIL[:, :], in_=wil_r[k, :, ns])
                    for bt in range(BT):
                        bs = slice(bt * P, (bt + 1) * P)
                        nc.tensor.matmul(ps[bt][:, :], XH[:, k, bs], WIH[:, :],
                                         start=(k == 0), stop=False)
                        nc.tensor.matmul(ps[bt][:, :], XL[:, k, bs], WIH[:, :],
                                         start=False, stop=False)
                        nc.tensor.matmul(ps[bt][:, :], XH[:, k, bs], WIL[:, :],
                                         start=False, stop=False)
                for k in range(KU):
                    WRH = pw.tile([P, 512], BF, name="WRH", tag="wrh")
                    WRL = pw.tile([P, 512], BF, name="WRL", tag="wrl")
                    nc.sync.dma_start(out=WRH[:, :], in_=wrh_r[k, :, ns])
                    nc.sync.dma_start(out=WRL[:, :], in_=wrl_r[k, :, ns])
                    for bt in range(BT):
                        bs = slice(bt * P, (bt + 1) * P)
                        nc.tensor.matmul(ps[bt][:, :], ZT[:, k, bs], WRH[:, :],
                                         start=False, stop=False)
                        nc.tensor.matmul(ps[bt][:, :], ZT[:, k, bs], WRL[:, :],
                                         start=False, stop=(k == KU - 1))

                for bt in range(BT):
                    bs = slice(bt * P, (bt + 1) * P)
                    zt = pa.tile([P, 512], F32, name="zt", tag="z")
                    vt = pa.tile([P, 512], F32, name="vt", tag="v")
                    rt = pa.tile([P, 512], F32, name="rt", tag="r")
                    bt_ = pa.tile([P, 512], F32, name="bt_", tag="b")
                    nc.sync.dma_start(out=zt[:, :], in_=z_in[bs, ns])
                    nc.sync.dma_start(out=vt[:, :], in_=v_in[bs, ns])
                    nc.sync.dma_start(out=rt[:, :], in_=r_in[bs, ns])
                    nc.sync.dma_start(out=bt_[:, :], in_=b_in[bs, ns])

                    nv = po.tile([P, 512], F32, name="nv", tag="nv")
                    nc.vector.scalar_tensor_tensor(
                        nv[:, :], vt[:, :], DECAY, ps[bt][:, :],
                        alu.mult, alu.add)
                    nbt = po.tile([P, 512], F32, name="nbt", tag="nb")
                    nc.vector.scalar_tensor_tensor(
                        nbt[:, :], bt_[:, :], DECAY_B, zt[:, :],
                        alu.mult, alu.add)

                    cond = pt.tile([P, 512], F32, name="cond", tag="cond")
                    if nb < NB // 2:
                        # LIF half: threshold is the constant 0.4
                        nc.vector.tensor_scalar(
                            cond[:, :], nv[:, :], THR, None, alu.is_gt)
                    else:
                        # ALIF half: spike iff 1.6*new_b - new_v < -0.4
                        d = pt.tile([P, 512], F32, name="d", tag="d")
                        nc.vector.scalar_tensor_tensor(
                            d[:, :], nbt[:, :], BETA, nv[:, :],
                            alu.mult, alu.subtract)
                        nc.vector.tensor_scalar(
                            cond[:, :], d[:, :], -THR, None, alu.is_lt)

                    nz = po.tile([P, 512], F32, name="nz", tag="nz")
                    nc.vector.scalar_tensor_tensor(
                        nz[:, :], rt[:, :], 0.0, cond[:, :], alu.is_le, alu.mult)
                    nzb_t = pt.tile([P, 512], BF, name="nzb_t", tag="nzb")
                    nc.vector.tensor_copy(nzb_t[:, :], nz[:, :])

                    t1 = pt.tile([P, 512], F32, name="t1", tag="t1")
                    nc.vector.tensor_scalar(
                        t1[:, :], rt[:, :], -1.0, 0.0, alu.add, alu.max)
                    nr = po.tile([P, 512], F32, name="nr", tag="nr")
                    nc.vector.scalar_tensor_tensor(
                        nr[:, :], nz[:, :], N_REF - 1.0, t1[:, :],
                        alu.mult, alu.add)

                    nc.sync.dma_start(out=new_v[bs, ns], in_=nv[:, :])
                    nc.sync.dma_start(out=new_b[bs, ns], in_=nbt[:, :])
                    nc.sync.dma_start(out=new_z[bs, ns], in_=nz[:, :])
                    nc.sync.dma_start(out=new_r[bs, ns], in_=nr[:, :])
                    nc.sync.dma_start(out=nzb[bs, ns], in_=nzb_t[:, :])

            # ---------- phase B: new_out = KAPPA*out + new_z @ w_out ----------
            NZT = []
            for bt in range(BT):
                bs = slice(bt * P, (bt + 1) * P)
                nzT = pnz.tile([P, KU, P], BF, name="nzT", tag="nzT")
                for k in range(KU):
                    nc.sync.dma_start(out=nzT[:, k, :],
                                      in_=nzb[bs, k * P:(k + 1) * P],
                                      transpose=True)
                NZT.append(nzT)

            for ob in range(OB):
                os_ = slice(ob * 512, (ob + 1) * 512)
                ps3 = []
                for bt in range(BT):
                    p3 = ppsB.tile([P, 512], F32, name="p3", tag="p3")
                    ps3.append(p3)
                for k in range(KU):
                    WOH = pwo.tile([P, 512], BF, name="WOH", tag="woh")
                    WOL = pwo.tile([P, 512], BF, name="WOL", tag="wol")
                    nc.sync.dma_start(out=WOH[:, :], in_=woh_r[k, :, os_])
                    nc.sync.dma_start(out=WOL[:, :], in_=wol_r[k, :, os_])
                    for bt in range(BT):
                        nc.tensor.matmul(ps3[bt][:, :], NZT[bt][:, k, :],
                                         WOH[:, :], start=(k == 0), stop=False)
                        nc.tensor.matmul(ps3[bt][:, :], NZT[bt][:, k, :],
                                         WOL[:, :], start=False,
                                         stop=(k == KU - 1))
                for bt in range(BT):
                    bs = slice(bt * P, (bt + 1) * P)
                    ot = pa2.tile([P, 512], F32, name="ot", tag="o")
                    nc.sync.dma_start(out=ot[:, :], in_=o_in[bs, os_])
                    no = pa2.tile([P, 512], F32, name="no", tag="no")
                    nc.vector.scalar_tensor_tensor(
                        no[:, :], ot[:, :], KAPPA, ps3[bt][:, :],
                        alu.mult, alu.add)
                    nc.sync.dma_start(out=new_out[bs, os_], in_=no[:, :])

    nc.compile()
    _cached_nc = nc
    return nc


def _split_bf16(a):
    hi = a.astype(BF16)
    lo = (a - hi.astype(np.float32)).astype(BF16)
    return hi, lo


def prepare_in_maps(inputs):
    """Host prep: splits/casts/transposes + per-core shard dicts."""
    x = np.asarray(inputs["x"], np.float32)
    z = np.asarray(inputs["z"], np.float32)
    w_in = np.asarray(inputs["w_in"], np.float32)
    w_rec = np.asarray(inputs["w_rec"], np.float32)
    w_out = np.asarray(inputs["w_out"], np.float32)

    # fold the -THR*z reset into w_rec's diagonal (z is 0/1)
    w_rec_m = w_rec.copy()
    w_rec_m[np.arange(UNITS), np.arange(UNITS)] -= THR

    wih, wil = _split_bf16(w_in)
    wrh, wrl = _split_bf16(w_rec_m)
    woh, wol = _split_bf16(w_out)
    xh, xl = _split_bf16(x)
    zb = z.astype(BF16)  # exact: z is 0/1

    v = np.asarray(inputs["v"], np.float32)
    r = np.asarray(inputs["r"], np.float32)
    out = np.asarray(inputs["out"], np.float32)
    b = np.asarray(inputs["b"], np.float32)

    in_maps = []
    for c in range(NCORES):
        sl = slice(c * BS, (c + 1) * BS)
        in_maps.append({
            "xhT": np.ascontiguousarray(xh[sl].T),
            "xlT": np.ascontiguousarray(xl[sl].T),
            "zT": np.ascontiguousarray(zb[sl].T),
            "z": z[sl], "v": v[sl], "r": r[sl], "b": b[sl], "o": out[sl],
            "wih": wih, "wil": wil, "wrh": wrh, "wrl": wrl,
            "woh": woh, "wol": wol,
        })
    return in_maps


def run_lsnn(inputs, trace=False):
    nc = build()
    in_maps = prepare_in_maps(inputs)
    res = bass_utils.run_bass_kernel_spmd(
        nc, in_maps, core_ids=list(range(NCORES)), trace=trace)

    def gather(name):
        return np.concatenate([res.results[c][name] for c in range(NCORES)], 0)

    outs = (gather("new_out"), gather("new_z"), gather("new_v"),
            gather("new_r"), gather("new_b"))
    return outs, res


def kernel(**inputs):
    outs, _ = run_lsnn(inputs, trace=False)
    return outs
